# revision 8
# baseline (speedup 1.0000x reference)
"""Trainium2 Bass kernel for a 2-layer IndRNN (adding-problem head).

Computation (matches the reference):
    pre1 = x @ W1.T + b1                    # [B,T,H], D=2
    h1   = scan over t: h = relu(pre1_t + u1*h)   (all steps kept)
    pre2 = h1 @ W2.T + b2                   # [B,T,H]
    h2T  = scan over t: h = relu(pre2_t + u2*h)   (last step only)
    out  = h2T @ Wf.T + bf                  # [B]

Sharding: data-parallel over batch across 8 NeuronCores (32 batch each).
All intermediates stay on-chip per core (no DRAM spills, no collectives).

Per-core layout is channel-major: [c_lo(128 partitions), (c_hi(4), t, b(32))].
Each scan advances one timestep per Vector-engine instruction pair:

    TT :  z  = w_prev + p_t           (tensor_tensor add)
    STT:  w  = (z max 0) * u_tile     (scalar_tensor_tensor: fused relu+mult)

w = u1*relu(z1) doubles as recurrence feedback and layer-2 matmul input
(the host folds 1/u1 into W2).  The two layers' scans are interleaved
instruction-by-instruction so independent chains overlap in the DVE
pipeline; layer 2 lags layer 1 by LAG chunks.

TensorE computes pre1 (K=2, weights stationary per c_hi block) and pre2
(W2 tiles stationary) in float32r (full PE rate at N=512); ScalarE drains
PSUM to SBUF with the per-channel bias fused into the activation.
"""

import os
import sys

for _p in ("/opt/trn_rl_repo", "/root/.axon_site", "/root/.axon_site/_ro/trn_rl_repo",
           "/root/.axon_site/_ro/pypackages"):
    if os.path.isdir(_p) and _p not in sys.path:
        sys.path.append(_p)

import numpy as np

B, T_FULL, D, H = 256, 2048, 2, 512
NCORES = 8
BL = B // NCORES          # 32 batch per core
TC = 32                   # timesteps per chunk
LAG = 2                   # layer-2 chunk lag behind layer 1

_COMPILED = {}


def _build(T):
    import contextlib
    from concourse import tile, bacc, mybir

    nchunks = T // TC
    nk = nchunks + LAG

    f32 = mybir.dt.float32
    f32r = mybir.dt.float32r
    Add = mybir.AluOpType.add
    Max = mybir.AluOpType.max
    Mult = mybir.AluOpType.mult
    Ident = mybir.ActivationFunctionType.Identity
    Relu = mybir.ActivationFunctionType.Relu

    CB = TC * BL            # free elems per (chunk, c_hi) = 1024
    NSUB = CB // 512        # 512-wide matmul subtiles per (chunk, c_hi)

    nc = bacc.Bacc("TRN2", target_bir_lowering=False, debug=False)

    x_d = nc.dram_tensor("x_sb", [2 * nchunks, CB], f32r, kind="ExternalInput").ap()
    w1_d = nc.dram_tensor("w1_rep", [2, 512], f32r, kind="ExternalInput").ap()
    b1_d = nc.dram_tensor("b1_col", [128, 4], f32, kind="ExternalInput").ap()
    u1_d = nc.dram_tensor("u1_tile", [128, 128], f32, kind="ExternalInput").ap()
    w2_d = nc.dram_tensor("w2t", [128, 2048], f32r, kind="ExternalInput").ap()
    b2_d = nc.dram_tensor("b2_col", [128, 4], f32, kind="ExternalInput").ap()
    u2_d = nc.dram_tensor("u2_tile", [128, 128], f32, kind="ExternalInput").ap()
    wf_d = nc.dram_tensor("wf_col", [128, 4], f32r, kind="ExternalInput").ap()
    out_d = nc.dram_tensor("out", [1, BL], f32, kind="ExternalOutput").ap()

    with tile.TileContext(nc) as tc:
        with contextlib.ExitStack() as ctx:
            consts = ctx.enter_context(tc.tile_pool(name="consts", bufs=1))
            p1_pool = ctx.enter_context(tc.tile_pool(name="p1", bufs=3))
            xs_pool = ctx.enter_context(tc.tile_pool(name="xs", bufs=4))
            w_pool = ctx.enter_context(tc.tile_pool(name="w", bufs=3))
            p2_pool = ctx.enter_context(tc.tile_pool(name="p2", bufs=4))
            st_pool = ctx.enter_context(tc.tile_pool(name="st", bufs=1))
            ps1 = ctx.enter_context(tc.tile_pool(name="ps1", bufs=2, space="PSUM"))
            ps2 = ctx.enter_context(tc.tile_pool(name="ps2", bufs=4, space="PSUM"))
            psf = ctx.enter_context(tc.tile_pool(name="psf", bufs=1, space="PSUM"))

            w1_sb = consts.tile([2, 512], f32r, name="w1_sbt")
            b1_sb = consts.tile([128, 4], f32, name="b1_sbt")
            u1_sb = consts.tile([128, 128], f32, name="u1_sbt")
            w2_sb = consts.tile([128, 2048], f32r, name="w2_sbt")
            b2_sb = consts.tile([128, 4], f32, name="b2_sbt")
            u2_sb = consts.tile([128, 128], f32, name="u2_sbt")
            wf_sb = consts.tile([128, 4], f32r, name="wf_sbt")
            for sb, dr in ((w1_sb, w1_d), (b1_sb, b1_d), (u1_sb, u1_d),
                           (b2_sb, b2_d), (u2_sb, u2_d), (wf_sb, wf_d),
                           (w2_sb, w2_d)):
                nc.gpsimd.dma_start(sb[:], dr[:])

            z1 = st_pool.tile([128, 128], f32, name="z1")
            z2 = st_pool.tile([128, 128], f32, name="z2")
            v2 = st_pool.tile([128, 128], f32, name="v2")
            zero = st_pool.tile([128, 128], f32, name="zero")
            nc.vector.memset(zero[:], 0.0)
            nc.vector.memset(v2[:], 0.0)

            p1_tiles, w_tiles, p2_tiles = {}, {}, {}

            def chunk_slot(tile_, i):
                # [128, 4, BL] view of a [128, 4*CB] chunk tile at step i
                return tile_[:].rearrange("p (c t b) -> p c t b", c=4, t=TC, b=BL)[:, :, i, :]

            def zview(t_):
                return t_[:].rearrange("p (c b) -> p c b", c=4, b=BL)

            def p1_matmul(k):
                xst = xs_pool.tile([2, CB], f32r, name=f"xst_{k}", tag="xst")
                nc.gpsimd.dma_start(xst[:], x_d[2 * k:2 * k + 2, :])
                pt = p1_pool.tile([128, 4 * CB], f32, name=f"p1t_{k}", tag="p1t")
                p1_tiles[k] = pt
                for c_hi in range(4):
                    for sub in range(NSUB):
                        ps = ps1.tile([128, 512], f32, name=f"p1ps_{k}_{c_hi}_{sub}", tag="p1ps")
                        lhsT = w1_sb[0:2, c_hi * 128:(c_hi + 1) * 128]
                        rhs = xst[0:2, sub * 512:(sub + 1) * 512]
                        nc.tensor.matmul(ps[:], lhsT, rhs,
                                         start=True, stop=True)
                        nc.scalar.activation(
                            pt[:, c_hi * CB + sub * 512: c_hi * CB + (sub + 1) * 512],
                            ps[:], Ident, bias=b1_sb[:, c_hi:c_hi + 1], scale=1.0)

            def w2_matmul(k):
                pt = p2_pool.tile([128, 4 * CB], f32, name=f"p2t_{k}", tag="p2t")
                p2_tiles[k] = pt
                wt = w_tiles[k]
                for g in range(4):
                    for sub in range(NSUB):
                        ps = ps2.tile([128, 512], f32, name=f"p2ps_{k}_{g}_{sub}", tag="p2ps")
                        for c in range(4):
                            lhsT = w2_sb[:, (c * 4 + g) * 128:(c * 4 + g + 1) * 128]
                            rhs = wt[:, c * CB + sub * 512: c * CB + (sub + 1) * 512]
                            nc.tensor.matmul(ps[:], lhsT, rhs,
                                             start=(c == 0), stop=(c == 3))
                        nc.scalar.activation(
                            pt[:, g * CB + sub * 512: g * CB + (sub + 1) * 512],
                            ps[:], Ident, bias=b2_sb[:, g:g + 1], scale=1.0)

            p1_matmul(0)
            if nchunks > 1:
                p1_matmul(1)

            for k in range(nk):
                if 1 <= k <= nchunks:
                    w2_matmul(k - 1)
                if k + 2 < nchunks:
                    p1_matmul(k + 2)

                l2k = k - LAG
                if k < nchunks:
                    wt = w_pool.tile([128, 4 * CB], f32r, name=f"wt_{k}", tag="wt")
                    w_tiles[k] = wt
                for i in range(TC):
                    if k < nchunks:
                        s = k * TC + i
                        if s == 0:
                            wprev = zview(zero)
                        elif i == 0:
                            wprev = chunk_slot(w_tiles[k - 1], TC - 1)
                        else:
                            wprev = chunk_slot(w_tiles[k], i - 1)
                        nc.vector.tensor_tensor(zview(z1), wprev,
                                                chunk_slot(p1_tiles[k], i), Add)
                    if l2k >= 0:
                        nc.vector.tensor_tensor(zview(z2), zview(v2),
                                                chunk_slot(p2_tiles[l2k], i), Add)
                    if k < nchunks:
                        nc.vector.scalar_tensor_tensor(chunk_slot(w_tiles[k], i), zview(z1),
                                                       0.0, zview(u1_sb), Max, Mult)
                    if l2k >= 0:
                        nc.vector.scalar_tensor_tensor(zview(v2), zview(z2),
                                                       0.0, zview(u2_sb), Max, Mult)

            hT = st_pool.tile([128, 128], f32r, name="hT")
            nc.scalar.activation(hT[:], z2[:], Relu, bias=0.0, scale=1.0)
            fin = psf.tile([1, BL], f32, name="fin")
            for g_hi in range(4):
                nc.tensor.matmul(fin[:], wf_sb[:, g_hi:g_hi + 1],
                                 hT[:, g_hi * BL:(g_hi + 1) * BL],
                                 start=(g_hi == 0), stop=(g_hi == 3))
            out_sb = st_pool.tile([1, BL], f32, name="out_sb")
            nc.scalar.activation(out_sb[:], fin[:], Ident, bias=0.0, scale=1.0)
            nc.gpsimd.dma_start(out_d[:], out_sb[:])

    nc.compile()
    return nc


def _prep_inputs(x, W1, u1, b1, W2, u2, b2, Wf, bf, T):
    f = np.float32
    u1c = np.where(np.abs(u1) < 1e-6, np.where(u1 >= 0, 1e-6, -1e-6), u1).astype(f)
    W2p = (W2 / u1c[None, :]).astype(f)

    nch = T // TC
    w1_rep = np.ascontiguousarray(W1.T).astype(f)   # [2, 512]
    b1_col = np.ascontiguousarray(b1.reshape(4, 128).T).astype(f)
    u1_tile = np.ascontiguousarray(
        np.broadcast_to(u1c.reshape(4, 128).T[:, :, None], (128, 4, BL)).reshape(128, 128))
    w2t = np.empty((128, 2048), f)
    for c_hi in range(4):
        for g_hi in range(4):
            blk = W2p[g_hi * 128:(g_hi + 1) * 128, c_hi * 128:(c_hi + 1) * 128]
            w2t[:, (c_hi * 4 + g_hi) * 128:(c_hi * 4 + g_hi + 1) * 128] = blk.T
    b2_col = np.ascontiguousarray(b2.reshape(4, 128).T).astype(f)
    u2_tile = np.ascontiguousarray(
        np.broadcast_to(u2.astype(f).reshape(4, 128).T[:, :, None], (128, 4, BL)).reshape(128, 128))
    wf_col = np.ascontiguousarray(Wf[0].reshape(4, 128).T).astype(f)

    in_maps = []
    for core in range(NCORES):
        xs = x[core * BL:(core + 1) * BL, :T, :]
        x_sb = np.ascontiguousarray(
            xs.reshape(BL, nch, TC, 2).transpose(1, 3, 2, 0).reshape(2 * nch, TC * BL)
        ).astype(f)  # row 2k+d, col t_lo*BL+b
        in_maps.append({
            "x_sb": x_sb, "w1_rep": w1_rep, "b1_col": b1_col, "u1_tile": u1_tile,
            "w2t": w2t, "b2_col": b2_col, "u2_tile": u2_tile, "wf_col": wf_col,
        })
    return in_maps


def kernel(x, W1, u1, b1, W2, u2, b2, Wf, bf, _T=None, _trace=False):
    x = np.asarray(x, np.float32)
    W1 = np.asarray(W1, np.float32); u1 = np.asarray(u1, np.float32)
    b1 = np.asarray(b1, np.float32); W2 = np.asarray(W2, np.float32)
    u2 = np.asarray(u2, np.float32); b2 = np.asarray(b2, np.float32)
    Wf = np.asarray(Wf, np.float32); bf = np.asarray(bf, np.float32)
    T = _T or x.shape[1]

    from concourse.bass_utils import run_bass_kernel_spmd

    if T not in _COMPILED:
        _COMPILED[T] = _build(T)
    nc = _COMPILED[T]

    in_maps = _prep_inputs(x, W1, u1, b1, W2, u2, b2, Wf, bf, T)
    res = run_bass_kernel_spmd(nc, in_maps, core_ids=list(range(NCORES)), trace=_trace)
    out = np.concatenate([res.results[i]["out"][0] for i in range(NCORES)]) + bf[0]
    kernel.last_exec_time_ns = res.exec_time_ns
    return out.astype(np.float32)


# revision 9
# speedup vs baseline: 1.0031x; 1.0031x over previous
"""Trainium2 Bass kernel for a 2-layer IndRNN (adding-problem head).

Computation (matches the reference):
    pre1 = x @ W1.T + b1                    # [B,T,H], D=2
    h1   = scan over t: h = relu(pre1_t + u1*h)   (all steps kept)
    pre2 = h1 @ W2.T + b2                   # [B,T,H]
    h2T  = scan over t: h = relu(pre2_t + u2*h)   (last step only)
    out  = h2T @ Wf.T + bf                  # [B]

Sharding: data-parallel over batch across 8 NeuronCores (32 batch each).
All intermediates stay on-chip per core (no DRAM spills, no collectives).

Per-core layout is channel-major: [c_lo(128 partitions), (c_hi(4), t, b(32))].
Each scan advances one timestep per Vector-engine instruction pair:

    TT :  z  = w_prev + p_t           (tensor_tensor add)
    STT:  w  = (z max 0) * u_tile     (scalar_tensor_tensor: fused relu+mult)

w = u1*relu(z1) doubles as recurrence feedback and layer-2 matmul input
(the host folds 1/u1 into W2).  The two layers' scans are interleaved
instruction-by-instruction so independent chains overlap in the DVE
pipeline; layer 2 lags layer 1 by LAG chunks.

TensorE computes pre1 (K=2, weights stationary per c_hi block) and pre2
(W2 tiles stationary) in float32r (full PE rate at N=512); ScalarE drains
PSUM to SBUF with the per-channel bias fused into the activation.
"""

import os
import sys

for _p in ("/opt/trn_rl_repo", "/root/.axon_site", "/root/.axon_site/_ro/trn_rl_repo",
           "/root/.axon_site/_ro/pypackages"):
    if os.path.isdir(_p) and _p not in sys.path:
        sys.path.append(_p)

import numpy as np

B, T_FULL, D, H = 256, 2048, 2, 512
NCORES = 8
BL = B // NCORES          # 32 batch per core
TC = 32                   # timesteps per chunk
LAG = 3                   # layer-2 chunk lag behind layer 1

_COMPILED = {}


def _build(T):
    import contextlib
    from concourse import tile, bacc, mybir

    nchunks = T // TC
    nk = nchunks + LAG

    f32 = mybir.dt.float32
    f32r = mybir.dt.float32r
    Add = mybir.AluOpType.add
    Max = mybir.AluOpType.max
    Mult = mybir.AluOpType.mult
    Ident = mybir.ActivationFunctionType.Identity
    Relu = mybir.ActivationFunctionType.Relu

    CB = TC * BL            # free elems per (chunk, c_hi) = 1024
    NSUB = CB // 512        # 512-wide matmul subtiles per (chunk, c_hi)

    nc = bacc.Bacc("TRN2", target_bir_lowering=False, debug=False)

    x_d = nc.dram_tensor("x_sb", [2 * nchunks, CB], f32r, kind="ExternalInput").ap()
    w1_d = nc.dram_tensor("w1_rep", [2, 512], f32r, kind="ExternalInput").ap()
    b1_d = nc.dram_tensor("b1_col", [128, 4], f32, kind="ExternalInput").ap()
    u1_d = nc.dram_tensor("u1_tile", [128, 128], f32, kind="ExternalInput").ap()
    w2_d = nc.dram_tensor("w2t", [128, 2048], f32r, kind="ExternalInput").ap()
    b2_d = nc.dram_tensor("b2_col", [128, 4], f32, kind="ExternalInput").ap()
    u2_d = nc.dram_tensor("u2_tile", [128, 128], f32, kind="ExternalInput").ap()
    wf_d = nc.dram_tensor("wf_col", [128, 4], f32r, kind="ExternalInput").ap()
    out_d = nc.dram_tensor("out", [1, BL], f32, kind="ExternalOutput").ap()

    with tile.TileContext(nc) as tc:
        with contextlib.ExitStack() as ctx:
            consts = ctx.enter_context(tc.tile_pool(name="consts", bufs=1))
            p1_pool = ctx.enter_context(tc.tile_pool(name="p1", bufs=3))
            xs_pool = ctx.enter_context(tc.tile_pool(name="xs", bufs=4))
            w_pool = ctx.enter_context(tc.tile_pool(name="w", bufs=3))
            p2_pool = ctx.enter_context(tc.tile_pool(name="p2", bufs=4))
            st_pool = ctx.enter_context(tc.tile_pool(name="st", bufs=1))
            ps1 = ctx.enter_context(tc.tile_pool(name="ps1", bufs=2, space="PSUM"))
            ps2 = ctx.enter_context(tc.tile_pool(name="ps2", bufs=4, space="PSUM"))
            psf = ctx.enter_context(tc.tile_pool(name="psf", bufs=1, space="PSUM"))

            w1_sb = consts.tile([2, 512], f32r, name="w1_sbt")
            b1_sb = consts.tile([128, 4], f32, name="b1_sbt")
            u1_sb = consts.tile([128, 128], f32, name="u1_sbt")
            w2_sb = consts.tile([128, 2048], f32r, name="w2_sbt")
            b2_sb = consts.tile([128, 4], f32, name="b2_sbt")
            u2_sb = consts.tile([128, 128], f32, name="u2_sbt")
            wf_sb = consts.tile([128, 4], f32r, name="wf_sbt")
            for sb, dr in ((w1_sb, w1_d), (b1_sb, b1_d)):
                nc.gpsimd.dma_start(sb[:], dr[:])

            z1 = st_pool.tile([128, 128], f32, name="z1")
            z2 = st_pool.tile([128, 128], f32, name="z2")
            v2 = st_pool.tile([128, 128], f32, name="v2")
            zero = st_pool.tile([128, 128], f32, name="zero")
            nc.vector.memset(zero[:], 0.0)
            nc.vector.memset(v2[:], 0.0)

            p1_tiles, w_tiles, p2_tiles = {}, {}, {}

            def chunk_slot(tile_, i):
                # [128, 4, BL] view of a [128, 4*CB] chunk tile at step i
                return tile_[:].rearrange("p (c t b) -> p c t b", c=4, t=TC, b=BL)[:, :, i, :]

            def zview(t_):
                return t_[:].rearrange("p (c b) -> p c b", c=4, b=BL)

            def p1_matmul(k):
                xst = xs_pool.tile([2, CB], f32r, name=f"xst_{k}", tag="xst")
                nc.gpsimd.dma_start(xst[:], x_d[2 * k:2 * k + 2, :])
                pt = p1_pool.tile([128, 4 * CB], f32, name=f"p1t_{k}", tag="p1t")
                p1_tiles[k] = pt
                for c_hi in range(4):
                    for sub in range(NSUB):
                        ps = ps1.tile([128, 512], f32, name=f"p1ps_{k}_{c_hi}_{sub}", tag="p1ps")
                        lhsT = w1_sb[0:2, c_hi * 128:(c_hi + 1) * 128]
                        rhs = xst[0:2, sub * 512:(sub + 1) * 512]
                        nc.tensor.matmul(ps[:], lhsT, rhs,
                                         start=True, stop=True)
                        nc.scalar.activation(
                            pt[:, c_hi * CB + sub * 512: c_hi * CB + (sub + 1) * 512],
                            ps[:], Ident, bias=b1_sb[:, c_hi:c_hi + 1], scale=1.0)

            def w2_matmul(k):
                pt = p2_pool.tile([128, 4 * CB], f32, name=f"p2t_{k}", tag="p2t")
                p2_tiles[k] = pt
                wt = w_tiles[k]
                for g in range(4):
                    for sub in range(NSUB):
                        ps = ps2.tile([128, 512], f32, name=f"p2ps_{k}_{g}_{sub}", tag="p2ps")
                        for c in range(4):
                            lhsT = w2_sb[:, (c * 4 + g) * 128:(c * 4 + g + 1) * 128]
                            rhs = wt[:, c * CB + sub * 512: c * CB + (sub + 1) * 512]
                            nc.tensor.matmul(ps[:], lhsT, rhs,
                                             start=(c == 0), stop=(c == 3))
                        nc.scalar.activation(
                            pt[:, g * CB + sub * 512: g * CB + (sub + 1) * 512],
                            ps[:], Ident, bias=b2_sb[:, g:g + 1], scale=1.0)

            p1_matmul(0)
            for sb, dr in ((u1_sb, u1_d), (b2_sb, b2_d), (u2_sb, u2_d),
                           (wf_sb, wf_d), (w2_sb, w2_d)):
                nc.gpsimd.dma_start(sb[:], dr[:])
            if nchunks > 1:
                p1_matmul(1)

            for k in range(nk):
                if 1 <= k <= nchunks:
                    w2_matmul(k - 1)
                if k + 2 < nchunks:
                    p1_matmul(k + 2)

                l2k = k - LAG
                if k < nchunks:
                    wt = w_pool.tile([128, 4 * CB], f32r, name=f"wt_{k}", tag="wt")
                    w_tiles[k] = wt
                for i in range(TC):
                    if k < nchunks:
                        s = k * TC + i
                        if s == 0:
                            wprev = zview(zero)
                        elif i == 0:
                            wprev = chunk_slot(w_tiles[k - 1], TC - 1)
                        else:
                            wprev = chunk_slot(w_tiles[k], i - 1)
                        nc.vector.tensor_tensor(zview(z1), wprev,
                                                chunk_slot(p1_tiles[k], i), Add)
                    if l2k >= 0:
                        nc.vector.tensor_tensor(zview(z2), zview(v2),
                                                chunk_slot(p2_tiles[l2k], i), Add)
                    if k < nchunks:
                        nc.vector.scalar_tensor_tensor(chunk_slot(w_tiles[k], i), zview(z1),
                                                       0.0, zview(u1_sb), Max, Mult)
                    if l2k >= 0:
                        nc.vector.scalar_tensor_tensor(zview(v2), zview(z2),
                                                       0.0, zview(u2_sb), Max, Mult)

            hT = st_pool.tile([128, 128], f32r, name="hT")
            nc.scalar.activation(hT[:], z2[:], Relu, bias=0.0, scale=1.0)
            fin = psf.tile([1, BL], f32, name="fin")
            for g_hi in range(4):
                nc.tensor.matmul(fin[:], wf_sb[:, g_hi:g_hi + 1],
                                 hT[:, g_hi * BL:(g_hi + 1) * BL],
                                 start=(g_hi == 0), stop=(g_hi == 3))
            out_sb = st_pool.tile([1, BL], f32, name="out_sb")
            nc.scalar.activation(out_sb[:], fin[:], Ident, bias=0.0, scale=1.0)
            nc.gpsimd.dma_start(out_d[:], out_sb[:])

    nc.compile()
    return nc


def _prep_inputs(x, W1, u1, b1, W2, u2, b2, Wf, bf, T):
    f = np.float32
    u1c = np.where(np.abs(u1) < 1e-6, np.where(u1 >= 0, 1e-6, -1e-6), u1).astype(f)
    W2p = (W2 / u1c[None, :]).astype(f)

    nch = T // TC
    w1_rep = np.ascontiguousarray(W1.T).astype(f)   # [2, 512]
    b1_col = np.ascontiguousarray(b1.reshape(4, 128).T).astype(f)
    u1_tile = np.ascontiguousarray(
        np.broadcast_to(u1c.reshape(4, 128).T[:, :, None], (128, 4, BL)).reshape(128, 128))
    w2t = np.empty((128, 2048), f)
    for c_hi in range(4):
        for g_hi in range(4):
            blk = W2p[g_hi * 128:(g_hi + 1) * 128, c_hi * 128:(c_hi + 1) * 128]
            w2t[:, (c_hi * 4 + g_hi) * 128:(c_hi * 4 + g_hi + 1) * 128] = blk.T
    b2_col = np.ascontiguousarray(b2.reshape(4, 128).T).astype(f)
    u2_tile = np.ascontiguousarray(
        np.broadcast_to(u2.astype(f).reshape(4, 128).T[:, :, None], (128, 4, BL)).reshape(128, 128))
    wf_col = np.ascontiguousarray(Wf[0].reshape(4, 128).T).astype(f)

    in_maps = []
    for core in range(NCORES):
        xs = x[core * BL:(core + 1) * BL, :T, :]
        x_sb = np.ascontiguousarray(
            xs.reshape(BL, nch, TC, 2).transpose(1, 3, 2, 0).reshape(2 * nch, TC * BL)
        ).astype(f)  # row 2k+d, col t_lo*BL+b
        in_maps.append({
            "x_sb": x_sb, "w1_rep": w1_rep, "b1_col": b1_col, "u1_tile": u1_tile,
            "w2t": w2t, "b2_col": b2_col, "u2_tile": u2_tile, "wf_col": wf_col,
        })
    return in_maps


def kernel(x, W1, u1, b1, W2, u2, b2, Wf, bf, _T=None, _trace=False):
    x = np.asarray(x, np.float32)
    W1 = np.asarray(W1, np.float32); u1 = np.asarray(u1, np.float32)
    b1 = np.asarray(b1, np.float32); W2 = np.asarray(W2, np.float32)
    u2 = np.asarray(u2, np.float32); b2 = np.asarray(b2, np.float32)
    Wf = np.asarray(Wf, np.float32); bf = np.asarray(bf, np.float32)
    T = _T or x.shape[1]

    from concourse.bass_utils import run_bass_kernel_spmd

    if T not in _COMPILED:
        _COMPILED[T] = _build(T)
    nc = _COMPILED[T]

    in_maps = _prep_inputs(x, W1, u1, b1, W2, u2, b2, Wf, bf, T)
    res = run_bass_kernel_spmd(nc, in_maps, core_ids=list(range(NCORES)), trace=_trace)
    out = np.concatenate([res.results[i]["out"][0] for i in range(NCORES)]) + bf[0]
    kernel.last_exec_time_ns = res.exec_time_ns
    return out.astype(np.float32)


# revision 10
# speedup vs baseline: 1.0052x; 1.0021x over previous
"""Trainium2 Bass kernel for a 2-layer IndRNN (adding-problem head).

Computation (matches the reference):
    pre1 = x @ W1.T + b1                    # [B,T,H], D=2
    h1   = scan over t: h = relu(pre1_t + u1*h)   (all steps kept)
    pre2 = h1 @ W2.T + b2                   # [B,T,H]
    h2T  = scan over t: h = relu(pre2_t + u2*h)   (last step only)
    out  = h2T @ Wf.T + bf                  # [B]

Sharding: data-parallel over batch across 8 NeuronCores (32 batch each).
All intermediates stay on-chip per core (no DRAM spills, no collectives).

Per-core layout is channel-major: [c_lo(128 partitions), (c_hi(4), t, b(32))].
Each scan advances one timestep per Vector-engine instruction pair:

    TT :  z  = w_prev + p_t           (tensor_tensor add)
    STT:  w  = (z max 0) * u_tile     (scalar_tensor_tensor: fused relu+mult)

w = u1*relu(z1) doubles as recurrence feedback and layer-2 matmul input
(the host folds 1/u1 into W2).  The two layers' scans are interleaved
instruction-by-instruction so independent chains overlap in the DVE
pipeline; layer 2 lags layer 1 by LAG chunks.

TensorE computes pre1 (K=2, weights stationary per c_hi block) and pre2
(W2 tiles stationary) in float32r (full PE rate at N=512); ScalarE drains
PSUM to SBUF with the per-channel bias fused into the activation.
"""

import os
import sys

for _p in ("/opt/trn_rl_repo", "/root/.axon_site", "/root/.axon_site/_ro/trn_rl_repo",
           "/root/.axon_site/_ro/pypackages"):
    if os.path.isdir(_p) and _p not in sys.path:
        sys.path.append(_p)

import numpy as np

B, T_FULL, D, H = 256, 2048, 2, 512
NCORES = 8
BL = B // NCORES          # 32 batch per core
TC = 32                   # timesteps per chunk
LAG = 2                   # layer-2 chunk lag behind layer 1

_COMPILED = {}


def _build(T):
    import contextlib
    from concourse import tile, bacc, mybir

    nchunks = T // TC
    nk = nchunks + LAG

    f32 = mybir.dt.float32
    f32r = mybir.dt.float32r
    Add = mybir.AluOpType.add
    Max = mybir.AluOpType.max
    Mult = mybir.AluOpType.mult
    Ident = mybir.ActivationFunctionType.Identity
    Relu = mybir.ActivationFunctionType.Relu

    CB = TC * BL            # free elems per (chunk, c_hi) = 1024
    NSUB = CB // 512        # 512-wide matmul subtiles per (chunk, c_hi)

    nc = bacc.Bacc("TRN2", target_bir_lowering=False, debug=False)

    x_d = nc.dram_tensor("x_sb", [2 * nchunks, CB], f32r, kind="ExternalInput").ap()
    w1_d = nc.dram_tensor("w1_rep", [2, 512], f32r, kind="ExternalInput").ap()
    b1_d = nc.dram_tensor("b1_col", [128, 4], f32, kind="ExternalInput").ap()
    u1_d = nc.dram_tensor("u1_tile", [128, 128], f32, kind="ExternalInput").ap()
    w2_d = nc.dram_tensor("w2t", [128, 2048], f32r, kind="ExternalInput").ap()
    b2_d = nc.dram_tensor("b2_col", [128, 4], f32, kind="ExternalInput").ap()
    u2_d = nc.dram_tensor("u2_tile", [128, 128], f32, kind="ExternalInput").ap()
    wf_d = nc.dram_tensor("wf_col", [128, 4], f32r, kind="ExternalInput").ap()
    out_d = nc.dram_tensor("out", [1, BL], f32, kind="ExternalOutput").ap()

    with tile.TileContext(nc) as tc:
        with contextlib.ExitStack() as ctx:
            consts = ctx.enter_context(tc.tile_pool(name="consts", bufs=1))
            p1_pool = ctx.enter_context(tc.tile_pool(name="p1", bufs=3))
            xs_pool = ctx.enter_context(tc.tile_pool(name="xs", bufs=4))
            w_pool = ctx.enter_context(tc.tile_pool(name="w", bufs=3))
            p2_pool = ctx.enter_context(tc.tile_pool(name="p2", bufs=4))
            st_pool = ctx.enter_context(tc.tile_pool(name="st", bufs=1))
            ps1 = ctx.enter_context(tc.tile_pool(name="ps1", bufs=2, space="PSUM"))
            ps2 = ctx.enter_context(tc.tile_pool(name="ps2", bufs=4, space="PSUM"))
            psf = ctx.enter_context(tc.tile_pool(name="psf", bufs=1, space="PSUM"))

            w1_sb = consts.tile([2, 512], f32r, name="w1_sbt")
            b1_sb = consts.tile([128, 4], f32, name="b1_sbt")
            u1_sb = consts.tile([128, 128], f32, name="u1_sbt")
            w2_sb = consts.tile([128, 2048], f32r, name="w2_sbt")
            b2_sb = consts.tile([128, 4], f32, name="b2_sbt")
            u2_sb = consts.tile([128, 128], f32, name="u2_sbt")
            wf_sb = consts.tile([128, 4], f32r, name="wf_sbt")
            for sb, dr in ((w1_sb, w1_d), (b1_sb, b1_d)):
                nc.gpsimd.dma_start(sb[:], dr[:])

            z1 = st_pool.tile([128, 128], f32, name="z1")
            z2 = st_pool.tile([128, 128], f32, name="z2")
            v2 = st_pool.tile([128, 128], f32, name="v2")
            zero = st_pool.tile([128, 128], f32, name="zero")
            nc.vector.memset(zero[:], 0.0)
            nc.vector.memset(v2[:], 0.0)

            p1_tiles, w_tiles, p2_tiles = {}, {}, {}

            def chunk_slot(tile_, i):
                # [128, 4, BL] view of a [128, 4*CB] chunk tile at step i
                return tile_[:].rearrange("p (c t b) -> p c t b", c=4, t=TC, b=BL)[:, :, i, :]

            def zview(t_):
                return t_[:].rearrange("p (c b) -> p c b", c=4, b=BL)

            def p1_matmul(k):
                xst = xs_pool.tile([2, CB], f32r, name=f"xst_{k}", tag="xst")
                nc.gpsimd.dma_start(xst[:], x_d[2 * k:2 * k + 2, :])
                pt = p1_pool.tile([128, 4 * CB], f32, name=f"p1t_{k}", tag="p1t")
                p1_tiles[k] = pt
                for sub in range(NSUB):
                    for c_hi in range(4):
                        ps = ps1.tile([128, 512], f32, name=f"p1ps_{k}_{c_hi}_{sub}", tag="p1ps")
                        lhsT = w1_sb[0:2, c_hi * 128:(c_hi + 1) * 128]
                        rhs = xst[0:2, sub * 512:(sub + 1) * 512]
                        nc.tensor.matmul(ps[:], lhsT, rhs,
                                         start=True, stop=True)
                        nc.scalar.activation(
                            pt[:, c_hi * CB + sub * 512: c_hi * CB + (sub + 1) * 512],
                            ps[:], Ident, bias=b1_sb[:, c_hi:c_hi + 1], scale=1.0)

            def w2_matmul(k):
                pt = p2_pool.tile([128, 4 * CB], f32, name=f"p2t_{k}", tag="p2t")
                p2_tiles[k] = pt
                wt = w_tiles[k]
                for g in range(4):
                    for sub in range(NSUB):
                        ps = ps2.tile([128, 512], f32, name=f"p2ps_{k}_{g}_{sub}", tag="p2ps")
                        for c in range(4):
                            lhsT = w2_sb[:, (c * 4 + g) * 128:(c * 4 + g + 1) * 128]
                            rhs = wt[:, c * CB + sub * 512: c * CB + (sub + 1) * 512]
                            nc.tensor.matmul(ps[:], lhsT, rhs,
                                             start=(c == 0), stop=(c == 3))
                        nc.scalar.activation(
                            pt[:, g * CB + sub * 512: g * CB + (sub + 1) * 512],
                            ps[:], Ident, bias=b2_sb[:, g:g + 1], scale=1.0)

            p1_matmul(0)
            for sb, dr in ((u1_sb, u1_d), (b2_sb, b2_d), (u2_sb, u2_d),
                           (wf_sb, wf_d), (w2_sb, w2_d)):
                nc.gpsimd.dma_start(sb[:], dr[:])
            if nchunks > 1:
                p1_matmul(1)

            for k in range(nk):
                if 1 <= k <= nchunks:
                    w2_matmul(k - 1)
                if k + 2 < nchunks:
                    p1_matmul(k + 2)

                l2k = k - LAG
                if k < nchunks:
                    wt = w_pool.tile([128, 4 * CB], f32r, name=f"wt_{k}", tag="wt")
                    w_tiles[k] = wt
                for i in range(TC):
                    if k < nchunks:
                        s = k * TC + i
                        if s == 0:
                            wprev = zview(zero)
                        elif i == 0:
                            wprev = chunk_slot(w_tiles[k - 1], TC - 1)
                        else:
                            wprev = chunk_slot(w_tiles[k], i - 1)
                        nc.vector.tensor_tensor(zview(z1), wprev,
                                                chunk_slot(p1_tiles[k], i), Add)
                    if l2k >= 0:
                        nc.vector.tensor_tensor(zview(z2), zview(v2),
                                                chunk_slot(p2_tiles[l2k], i), Add)
                    if k < nchunks:
                        nc.vector.scalar_tensor_tensor(chunk_slot(w_tiles[k], i), zview(z1),
                                                       0.0, zview(u1_sb), Max, Mult)
                    if l2k >= 0:
                        nc.vector.scalar_tensor_tensor(zview(v2), zview(z2),
                                                       0.0, zview(u2_sb), Max, Mult)

            hT = st_pool.tile([128, 128], f32r, name="hT")
            nc.scalar.activation(hT[:], z2[:], Relu, bias=0.0, scale=1.0)
            fin = psf.tile([1, BL], f32, name="fin")
            for g_hi in range(4):
                nc.tensor.matmul(fin[:], wf_sb[:, g_hi:g_hi + 1],
                                 hT[:, g_hi * BL:(g_hi + 1) * BL],
                                 start=(g_hi == 0), stop=(g_hi == 3))
            out_sb = st_pool.tile([1, BL], f32, name="out_sb")
            nc.scalar.activation(out_sb[:], fin[:], Ident, bias=0.0, scale=1.0)
            nc.gpsimd.dma_start(out_d[:], out_sb[:])

    nc.compile()
    return nc


def _prep_inputs(x, W1, u1, b1, W2, u2, b2, Wf, bf, T):
    f = np.float32
    u1c = np.where(np.abs(u1) < 1e-6, np.where(u1 >= 0, 1e-6, -1e-6), u1).astype(f)
    W2p = (W2 / u1c[None, :]).astype(f)

    nch = T // TC
    w1_rep = np.ascontiguousarray(W1.T).astype(f)   # [2, 512]
    b1_col = np.ascontiguousarray(b1.reshape(4, 128).T).astype(f)
    u1_tile = np.ascontiguousarray(
        np.broadcast_to(u1c.reshape(4, 128).T[:, :, None], (128, 4, BL)).reshape(128, 128))
    w2t = np.empty((128, 2048), f)
    for c_hi in range(4):
        for g_hi in range(4):
            blk = W2p[g_hi * 128:(g_hi + 1) * 128, c_hi * 128:(c_hi + 1) * 128]
            w2t[:, (c_hi * 4 + g_hi) * 128:(c_hi * 4 + g_hi + 1) * 128] = blk.T
    b2_col = np.ascontiguousarray(b2.reshape(4, 128).T).astype(f)
    u2_tile = np.ascontiguousarray(
        np.broadcast_to(u2.astype(f).reshape(4, 128).T[:, :, None], (128, 4, BL)).reshape(128, 128))
    wf_col = np.ascontiguousarray(Wf[0].reshape(4, 128).T).astype(f)

    in_maps = []
    for core in range(NCORES):
        xs = x[core * BL:(core + 1) * BL, :T, :]
        x_sb = np.ascontiguousarray(
            xs.reshape(BL, nch, TC, 2).transpose(1, 3, 2, 0).reshape(2 * nch, TC * BL)
        ).astype(f)  # row 2k+d, col t_lo*BL+b
        in_maps.append({
            "x_sb": x_sb, "w1_rep": w1_rep, "b1_col": b1_col, "u1_tile": u1_tile,
            "w2t": w2t, "b2_col": b2_col, "u2_tile": u2_tile, "wf_col": wf_col,
        })
    return in_maps


def kernel(x, W1, u1, b1, W2, u2, b2, Wf, bf, _T=None, _trace=False):
    x = np.asarray(x, np.float32)
    W1 = np.asarray(W1, np.float32); u1 = np.asarray(u1, np.float32)
    b1 = np.asarray(b1, np.float32); W2 = np.asarray(W2, np.float32)
    u2 = np.asarray(u2, np.float32); b2 = np.asarray(b2, np.float32)
    Wf = np.asarray(Wf, np.float32); bf = np.asarray(bf, np.float32)
    T = _T or x.shape[1]

    from concourse.bass_utils import run_bass_kernel_spmd

    if T not in _COMPILED:
        _COMPILED[T] = _build(T)
    nc = _COMPILED[T]

    in_maps = _prep_inputs(x, W1, u1, b1, W2, u2, b2, Wf, bf, T)
    res = run_bass_kernel_spmd(nc, in_maps, core_ids=list(range(NCORES)), trace=_trace)
    out = np.concatenate([res.results[i]["out"][0] for i in range(NCORES)]) + bf[0]
    kernel.last_exec_time_ns = res.exec_time_ns
    return out.astype(np.float32)
